# revision 1
# baseline (speedup 1.0000x reference)
"""GCN-GRU encoder (DCRNN-style) on 8 TRN2 NeuronCores, data-parallel over B.

Layouts per core (B_loc=4, N=512, H=64, C=2, K=3, T=12):
  HT[b]  [64, 512]  fp32  feature-major GRU state (persistent accumulator)
  HN[j]  [128, 256] f32r  node-major h, cols = b*64+f   (agg lhsT, b-paired)
  CN[j]  [128, 256] f32r  node-major r*h
  gt[k][j] [128,512] f32r G[k].T j-tile                  (agg rhs)
  AGS    [128, 512] f32r  feature-major aggregates, rows = (pair member, f)
  XA     [102,6144] f32r  x-part agg, rows b*32+(k*2+c), cols t*512+i
Gate/update pre-activations accumulate in PSUM feature-major [o, i]; sigmoid/
tanh on ACT with per-partition bias; GRU elementwise on DVE in fp32.
"""
import os
import numpy as np

import concourse.bass as bass
import concourse.tile as tile
from concourse import mybir
from concourse.bass_utils import run_bass_kernel_spmd

dt = mybir.dt
AF = mybir.ActivationFunctionType
ALU = mybir.AluOpType

B, T, N, C, H, K = 32, 12, 512, 2, 64, 3
NCORES = 8
BL = B // NCORES          # 4 batches per core
NT = N // 128             # 4 partition tiles of the node dim
P = C + H                 # 66
MM_DT = dt.float32 if os.environ.get("KERNEL_MM_DT") == "f32" else dt.float32r

_waitsplit_ctr = [0]


def _split_excess_waits(nc, max_waits=1):
    """This walrus build allows only `max_waits` semaphore waits per
    instruction; hoist the excess onto preceding same-engine NoOps."""
    for f in nc.m.functions:
        for blk in f.blocks:
            new = []
            for inst in blk.instructions:
                si = inst.sync_info
                if si is not None and len(si.on_wait) > max_waits:
                    waits = list(si.on_wait)
                    head, tail = waits[:-max_waits], waits[-max_waits:]
                    for s in range(0, len(head), max_waits):
                        nop = mybir.InstNoOp(
                            name=f"I-waitsplit-{_waitsplit_ctr[0]}", ins=[], outs=[])
                        _waitsplit_ctr[0] += 1
                        nop.engine = inst.engine
                        nop.sync_info = mybir.SyncInfo(
                            on_wait=list(head[s:s + max_waits]), on_update=[])
                        new.append(nop)
                    inst.sync_info = mybir.SyncInfo(
                        on_wait=list(tail), on_update=list(si.on_update))
                new.append(inst)
            blk.instructions[:] = new


DBG_T = 1


def _build_nc(debug=False):
    assert not debug, "v2 has no debug dumps"
    nc = bass.Bass()
    f32 = dt.float32
    dbg = {}
    if debug:
        dbg["DXA"] = nc.declare_dram_parameter("DXA", [102, T * N], f32, isOutput=True)
        dbg["DAGS"] = nc.declare_dram_parameter("DAGS", [128, N], f32, isOutput=True)
        dbg["DZ"] = nc.declare_dram_parameter("DZ", [H, N], f32, isOutput=True)
        dbg["DR"] = nc.declare_dram_parameter("DR", [H, N], f32, isOutput=True)
        dbg["DCN"] = nc.declare_dram_parameter("DCN", [128, 2 * H], f32, isOutput=True)
        dbg["DAGS2"] = nc.declare_dram_parameter("DAGS2", [128, N], f32, isOutput=True)
        dbg["DHC"] = nc.declare_dram_parameter("DHC", [H, N], f32, isOutput=True)
        dbg["DHT1"] = nc.declare_dram_parameter("DHT1", [H, N], f32, isOutput=True)
        dbg["DHN1"] = nc.declare_dram_parameter("DHN1", [128, 2 * H], f32, isOutput=True)
    GT_d = nc.declare_dram_parameter("GT", [K, N, N], MM_DT, isOutput=False)
    XN_d = nc.declare_dram_parameter("XN", [N, BL * T * C], MM_DT, isOutput=False)
    H0_d = nc.declare_dram_parameter("H0", [N, BL * H], MM_DT, isOutput=False)
    H0T_d = nc.declare_dram_parameter("H0T", [BL * H, N], f32, isOutput=False)
    WHG01_d = nc.declare_dram_parameter("WHG01", [128, 128], MM_DT, isOutput=False)
    WHG2_d = nc.declare_dram_parameter("WHG2", [128, 128], MM_DT, isOutput=False)
    WHU01_d = nc.declare_dram_parameter("WHU01", [128, 64], MM_DT, isOutput=False)
    WHU2_d = nc.declare_dram_parameter("WHU2", [128, 64], MM_DT, isOutput=False)
    WXG_d = nc.declare_dram_parameter("WXG", [102, 128], MM_DT, isOutput=False)
    WXU_d = nc.declare_dram_parameter("WXU", [102, 64], MM_DT, isOutput=False)
    BG_d = nc.declare_dram_parameter("BG", [128, 1], f32, isOutput=False)
    BU_d = nc.declare_dram_parameter("BU", [64, 1], f32, isOutput=False)
    EYE_d = nc.declare_dram_parameter("EYE", [64, 64], f32, isOutput=False)
    HOUT_d = nc.declare_dram_parameter("HOUT", [BL * H, N], f32, isOutput=True)
    XAS_d = nc.dram_tensor("XAS_scratch", [K, BL * T * C, N], MM_DT)

    with tile.TileContext(nc) as tc:
        with tc.tile_pool(name="const", bufs=1) as cst, \
             tc.tile_pool(name="ags", bufs=16) as agsp, \
             tc.tile_pool(name="zs", bufs=1) as zsp, \
             tc.tile_pool(name="hcs", bufs=2) as hcsp, \
             tc.tile_pool(name="candh", bufs=2) as chp, \
             tc.tile_pool(name="tmp", bufs=1) as tmpp, \
             tc.tile_pool(name="ht", bufs=2) as htp, \
             tc.tile_pool(name="hn", bufs=2) as hnp, \
             tc.tile_pool(name="cn", bufs=2) as cnp, \
             tc.tile_pool(name="aggps", bufs=2, space="PSUM") as aggps, \
             tc.tile_pool(name="zrps", bufs=3, space="PSUM") as zrps, \
             tc.tile_pool(name="trps", bufs=1, space="PSUM") as trps:

            def round_in(dst_shape, src_ap, tag):
                """Direct DMA of host-pre-rounded f32r data."""
                d = cst.tile(dst_shape, MM_DT, tag=tag)
                nc.sync.dma_start(d[:], src_ap)
                return d

            # ---- constants / inputs ----
            gt = [[round_in([128, N], GT_d[k, j * 128:(j + 1) * 128, :],
                            f"gt{k}{j}") for j in range(NT)] for k in range(K)]
            xn = [round_in([128, BL * T * C], XN_d[j * 128:(j + 1) * 128, :],
                           f"xn{j}") for j in range(NT)]
            whg01 = round_in([128, 128], WHG01_d[:], "whg01")
            whg2 = round_in([128, 128], WHG2_d[:], "whg2")
            whu01 = round_in([128, 64], WHU01_d[:], "whu01")
            whu2 = round_in([128, 64], WHU2_d[:], "whu2")
            wxg = round_in([102, 128], WXG_d[:], "wxg")
            wxu = round_in([102, 64], WXU_d[:], "wxu")
            bgz = cst.tile([64, 1], dt.float32, tag="bgz")
            nc.sync.dma_start(bgz[:], BG_d[0:64, :])
            bgr = cst.tile([64, 1], dt.float32, tag="bgr")
            nc.sync.dma_start(bgr[:], BG_d[64:128, :])
            bu = cst.tile([64, 1], dt.float32, tag="bu")
            nc.sync.dma_start(bu[:], BU_d[:])
            eye = cst.tile([64, 64], dt.float32, tag="eye")
            nc.sync.dma_start(eye[:], EYE_d[:])

            # initial node-major h (f32r), split per batch-pair so the two
            # pair pipelines share no tiles (lets steps overlap)
            HN = [[None] * NT for _ in range(2)]
            for j in range(NT):
                for p in range(2):
                    hn0 = hnp.tile([128, 2 * H], MM_DT, tag=f"hn{p}{j}")
                    nc.sync.dma_start(
                        hn0[:], H0_d[j * 128:(j + 1) * 128,
                                     p * 128:(p + 1) * 128])
                    HN[p][j] = hn0
            HT = []
            for b in range(BL):
                ht0 = htp.tile([H, N], dt.float32, tag=f"ht{b}")
                nc.sync.dma_start(ht0[:], H0T_d[b * H:(b + 1) * H, :])
                HT.append(ht0)

            # ---- x aggregation precompute: XAS[k] = (G_k @ x).T for all (b,t,c) ----
            XA = cst.tile([102, T * N], MM_DT, tag="xa")
            for k in range(K):
                ps = aggps.tile([BL * T * C, N], dt.float32, tag="agg0")
                for j in range(NT):
                    nc.tensor.matmul(ps[:], xn[j][:], gt[k][j][:],
                                     start=(j == 0), stop=(j == NT - 1))
                xas = cst.tile([BL * T * C, N], MM_DT, tag=f"xas{k}")
                nc.vector.tensor_copy(xas[:], ps[:])
                nc.sync.dma_start(XAS_d[k], xas[:])
            # scatter rows (k;b,t,c) -> XA rows b*32+k*2+c, cols t*512+i,
            # via DRAM so every stride is a plain memory stride
            srcv = XAS_d.rearrange("k (b t c) i -> b c k t i", b=BL, c=C)
            for b in range(BL):
                for c in range(C):
                    row0 = b * 32 + c
                    dst = XA[row0:row0 + C * (K - 1) + 1:C, :].rearrange(
                        "p (t i) -> p t i", t=T)
                    nc.sync.dma_start(dst, srcv[b, c])


            def aggregate_pair(srcN_p, p):
                """Feature-major aggregates for one batch pair: per-b packed
                [k0;k1] tiles [128,512] plus a pair-layout k2 tile.  The
                PSUM->SBUF copies alternate DVE/ACT to balance engines."""
                ci = p
                pss = []
                for k in range(K):
                    ps = aggps.tile([128, N], dt.float32, tag=f"agg{p}")
                    for j in range(NT):
                        nc.tensor.matmul(
                            ps[:], srcN_p[j][:], gt[k][j][:],
                            start=(j == 0), stop=(j == NT - 1))
                    pss.append(ps)
                    if k == 1:
                        # drain k0/k1 psums into per-b packed tiles asap
                        nonlocal_a01 = []
                        for bi in range(2):
                            t01 = agsp.tile([128, N], MM_DT, tag="ags")
                            for kk in range(2):
                                src_ap = pss[kk][bi * 64:(bi + 1) * 64, :]
                                dst_ap = t01[kk * 64:(kk + 1) * 64, :]
                                if ci % 2 == 0:
                                    i_ = nc.vector.tensor_copy(dst_ap, src_ap)
                                else:
                                    i_ = nc.scalar.copy(dst_ap, src_ap)
                                i_.ins.bass_priority = -20
                                ci += 1
                            nonlocal_a01.append(t01)
                a2 = agsp.tile([128, N], MM_DT, tag="ags")
                i_ = nc.scalar.copy(a2[:], pss[2][:])
                i_.ins.bass_priority = -20
                return nonlocal_a01, a2

            def transpose_pair(ft0, ft1, p, dst_tag, pool):
                """Two [64,512] fp32 -> node-major [128, 128] f32r per j."""
                out = []
                for j in range(NT):
                    trp = trps.tile([128, 2 * H], dt.float32, tag="tr")
                    nc.tensor.transpose(trp[:, 0:H],
                                        ft0[:, j * 128:(j + 1) * 128], eye[:])
                    nc.tensor.transpose(trp[:, H:2 * H],
                                        ft1[:, j * 128:(j + 1) * 128], eye[:])
                    d = pool.tile([128, 2 * H], MM_DT, tag=f"{dst_tag}{p}{j}")
                    nc.vector.tensor_copy(d[:], trp[:])
                    out.append(d)
                return out

            if debug:
                nc.sync.dma_start(dbg["DXA"][:], XA[:].bitcast(f32))

            # ---- time loop: 2-stream software pipeline.  Stream p=1 runs
            # 3 phases behind p=0, so each stream's heavy PE phases (the
            # aggregations) cover the other stream's activation/elementwise
            # latency chains ----
            HF = N // 2
            st = [dict(), dict()]

            def ph_agg_gate(p, t):
                st[p]["agsG"] = aggregate_pair(HN[p], p)

            def ph_gate_w(p, t):
                agsG = st[p]["agsG"]
                zrt, zs, rs = [], [None] * 2, [None] * 2
                for bi in range(2):
                    b = p * 2 + bi
                    zr = zrps.tile([128, N], dt.float32, tag="zr")
                    nc.tensor.matmul(
                        zr[:], wxg[b * 32:b * 32 + K * C, :],
                        XA[b * 32:b * 32 + K * C, t * N:(t + 1) * N],
                        start=True, stop=False, tile_position=(b * 32, 0))
                    zrt.append(zr)
                for bi in range(2):
                    nc.tensor.matmul(zrt[bi][:], whg01[:], agsG[0][bi][:],
                                     start=False, stop=False)
                for bi in range(2):
                    b = p * 2 + bi
                    half = bi * 64
                    zr = zrt[bi]
                    nc.tensor.matmul(zr[:], whg2[half:half + 64, :],
                                     agsG[1][half:half + 64, :],
                                     start=False, stop=True)
                    r = zsp.tile([H, N], dt.float32, tag=f"rs{b}")
                    for hh in range(2):
                        sl = slice(hh * HF, (hh + 1) * HF)
                        nc.scalar.activation(r[:, sl], zr[64:128, sl],
                                             AF.Sigmoid, bias=bgr[:])
                    rs[bi] = r
                    z = zsp.tile([H, N], dt.float32, tag=f"zs{b}")
                    nc.scalar.activation(z[:], zr[0:64, :], AF.Sigmoid,
                                         bias=bgz[:])
                    zs[bi] = z
                st[p]["zs"], st[p]["rs"] = zs, rs

            def ph_rt(p, t):
                zs, rs = st[p]["zs"], st[p]["rs"]
                candh, zh = [None] * 2, [None] * 2
                for bi in range(2):
                    b = p * 2 + bi
                    ch = chp.tile([H, N], dt.float32, tag=f"ch{b}")
                    for hh in range(2):
                        sl = slice(hh * HF, (hh + 1) * HF)
                        nc.vector.tensor_tensor(ch[:, sl], rs[bi][:, sl],
                                                HT[b][:, sl], ALU.mult)
                    candh[bi] = ch
                    # (1-z)*h = h - z*h, off the critical path on gpsimd
                    zhb = tmpp.tile([H, N], dt.float32, tag=f"zh{b}")
                    nc.gpsimd.tensor_tensor(zhb[:], zs[bi][:], HT[b][:],
                                            ALU.mult)
                    zh2 = tmpp.tile([H, N], dt.float32, tag=f"zi{b}")
                    nc.gpsimd.tensor_tensor(zh2[:], HT[b][:], zhb[:],
                                            ALU.subtract)
                    zh[bi] = zh2
                st[p]["zh"] = zh
                st[p]["CN"] = transpose_pair(candh[0][:], candh[1][:],
                                             p, "cn", cnp)

            def ph_agg_cand(p, t):
                st[p]["agsU"] = aggregate_pair(st[p]["CN"], p)

            def ph_upd_w(p, t):
                agsU = st[p]["agsU"]
                hct, hcs = [], [None] * 2
                for bi in range(2):
                    b = p * 2 + bi
                    hc = zrps.tile([H, N], dt.float32, tag="zr")
                    nc.tensor.matmul(
                        hc[:], wxu[b * 32:b * 32 + K * C, :],
                        XA[b * 32:b * 32 + K * C, t * N:(t + 1) * N],
                        start=True, stop=False, tile_position=(b * 32, 0))
                    hct.append(hc)
                for bi in range(2):
                    nc.tensor.matmul(hct[bi][:], whu01[:], agsU[0][bi][:],
                                     start=False, stop=False)
                for bi in range(2):
                    b = p * 2 + bi
                    half = bi * 64
                    hc = hct[bi]
                    nc.tensor.matmul(hc[:], whu2[half:half + 64, :],
                                     agsU[1][half:half + 64, :],
                                     start=False, stop=True)
                    hs = hcsp.tile([H, N], dt.float32, tag=f"hcs{b}")
                    for hh in range(2):
                        sl = slice(hh * HF, (hh + 1) * HF)
                        nc.scalar.activation(hs[:, sl], hc[:, sl], AF.Tanh,
                                             bias=bu[:])
                    hcs[bi] = hs
                st[p]["hcs"] = hcs

            def ph_update(p, t):
                zs, zh, hcs = st[p]["zs"], st[p]["zh"], st[p]["hcs"]
                for bi in range(2):
                    b = p * 2 + bi
                    zhc = tmpp.tile([H, N], dt.float32, tag=f"zd{b}")
                    htn = htp.tile([H, N], dt.float32, tag=f"ht{b}")
                    for hh in range(2):
                        sl = slice(hh * HF, (hh + 1) * HF)
                        nc.vector.tensor_tensor(zhc[:, sl], zs[bi][:, sl],
                                                hcs[bi][:, sl], ALU.mult)
                        eng = nc.vector if hh == 0 else nc.gpsimd
                        eng.tensor_tensor(htn[:, sl], zh[bi][:, sl],
                                          zhc[:, sl], ALU.add)
                    HT[b] = htn
                if t < T - 1:
                    HN[p] = transpose_pair(HT[p * 2][:], HT[p * 2 + 1][:],
                                           p, "hn", hnp)

            PHASES = [ph_agg_gate, ph_gate_w, ph_rt, ph_agg_cand,
                      ph_upd_w, ph_update]
            OFF = 2
            for tick in range(6 * T + OFF):
                for p in range(2):
                    local = tick - OFF * p
                    if 0 <= local < 6 * T:
                        t, ph = divmod(local, 6)
                        PHASES[ph](p, t)

            for b in range(BL):
                nc.sync.dma_start(HOUT_d[b * H:(b + 1) * H, :], HT[b][:])

    _split_excess_waits(nc, max_waits=1)
    return nc


_NC_CACHE = {}


def _get_nc():
    if "nc" not in _NC_CACHE:
        _NC_CACHE["nc"] = _build_nc()
    return _NC_CACHE["nc"]


def _round_f32r(a):
    """Bit-exact emulation of the device fp32r rounding (RNE to 11 explicit
    mantissa bits; verified against DVE-rounded output on hardware)."""
    if MM_DT is not dt.float32r:
        return np.ascontiguousarray(a, dtype=np.float32)
    v = np.ascontiguousarray(a, dtype=np.float32).view(np.uint32)
    lower = v & np.uint32(0xFFF)
    upper = v & ~np.uint32(0xFFF)
    up = (lower > 0x800) | ((lower == 0x800) & (((v >> np.uint32(12)) & 1) == 1))
    return (upper + np.where(up, np.uint32(0x1000), np.uint32(0))).view(np.float32)


def _host_prep(G, x_seq, init_h, W_gate, b_gate, W_update, b_update):
    f32 = np.float32
    GT = np.ascontiguousarray(G.transpose(0, 2, 1)).astype(f32)
    WG3 = W_gate.reshape(K, P, 2 * H).astype(f32)
    WU3 = W_update.reshape(K, P, H).astype(f32)
    WHG01 = np.concatenate([WG3[0, C:, :], WG3[1, C:, :]], axis=0)
    WHG2 = np.concatenate([WG3[2, C:, :], WG3[2, C:, :]], axis=0)
    WHU01 = np.concatenate([WU3[0, C:, :], WU3[1, C:, :]], axis=0)
    WHU2 = np.concatenate([WU3[2, C:, :], WU3[2, C:, :]], axis=0)
    WXG = np.zeros((102, 128), f32)
    WXU = np.zeros((102, 64), f32)
    for b in range(BL):
        for k in range(K):
            WXG[b * 32 + k * C: b * 32 + (k + 1) * C, :] = WG3[k, :C, :]
            WXU[b * 32 + k * C: b * 32 + (k + 1) * C, :] = WU3[k, :C, :]
    shared = {
        "GT": _round_f32r(GT),
        "WHG01": _round_f32r(WHG01), "WHG2": _round_f32r(WHG2),
        "WHU01": _round_f32r(WHU01), "WHU2": _round_f32r(WHU2),
        "WXG": _round_f32r(WXG), "WXU": _round_f32r(WXU),
        "BG": b_gate.reshape(128, 1).astype(f32),
        "BU": b_update.reshape(64, 1).astype(f32),
        "EYE": np.eye(64, dtype=f32),
    }
    in_maps = []
    for c in range(NCORES):
        b0 = c * BL
        xs = x_seq[b0:b0 + BL].astype(f32)              # [4, 12, 512, 2]
        h0 = init_h[b0:b0 + BL].astype(f32)             # [4, 512, 64]
        m = dict(shared)
        m["XN"] = _round_f32r(
            xs.transpose(2, 0, 1, 3).reshape(N, BL * T * C))
        m["H0"] = _round_f32r(
            h0.transpose(1, 0, 2).reshape(N, BL * H))
        m["H0T"] = np.ascontiguousarray(
            h0.transpose(0, 2, 1)).reshape(BL * H, N)
        in_maps.append(m)
    return in_maps


def _run(inputs, trace=False):
    nc = _get_nc()
    in_maps = _host_prep(**inputs)
    res = run_bass_kernel_spmd(nc, in_maps, list(range(NCORES)), trace=trace)
    outs = []
    for c in range(NCORES):
        hout = res.results[c]["HOUT"]                   # [256, 512]
        outs.append(hout.reshape(BL, H, N).transpose(0, 2, 1))
    full = np.concatenate(outs, axis=0).astype(np.float32)
    return full, res


def kernel(G, x_seq, init_h, W_gate, b_gate, W_update, b_update):
    full, _ = _run(dict(G=G, x_seq=x_seq, init_h=init_h, W_gate=W_gate,
                        b_gate=b_gate, W_update=W_update, b_update=b_update))
    return full



# revision 83
# speedup vs baseline: 1.7214x; 1.7214x over previous
"""GCN-GRU encoder (DCRNN-style) on 8 TRN2 NeuronCores, data-parallel over B.

v3: fp8e4 DoubleRow aggregation + pair-packed bf16 elementwise.

Per-core layouts (BL=4 batches as 2 pairs p, N=512, H=64, C=2, K=3, T=12):
  HT[p]   [128, 512] bf16  pair feature-major state, rows bi*64+f
  HN[p]   [128, 512] fp8   node-major 8*h, cols j*128+(bi*64+f)  (DR agg lhsT)
  GTP[k]  [128, 2048] fp8  16*G^T, cols jp*1024+jj*512+dst       (DR agg rhs)
  AG01    [128, 1024] fp8  128*(G_k h) hops k0|k1, rows (bi,f)   (DR weight rhs)
  AG2     [128, 512] bf16  128*(G_2 h)
  XA      [102, T*512] bf16 x-part aggregates (exact G), rows b*32+k*2+c
Weight matmuls accumulate S=4096-scaled pre-activations in PSUM; sigmoid/tanh
apply scale=1/4096 + bias on ACT.  Gate z/r column order is swapped per batch
parity so every elementwise op is partition-base aligned.
"""
import os
import numpy as np
import ml_dtypes

import concourse.bass as bass
import concourse.tile as tile
from concourse import mybir
from concourse.bass_utils import run_bass_kernel_spmd

dt = mybir.dt
AF = mybir.ActivationFunctionType
ALU = mybir.AluOpType
PM = mybir.MatmulPerfMode

B, T, N, C, H, K = 32, 12, 512, 2, 64, 3
NCORES = 8
BL = B // NCORES          # 4 batches per core -> 2 pairs
NT = N // 128
P = C + H

S_G, S_H, S_W = 16.0, 8.0, 32.0
S_TOT = S_G * S_H * S_W   # 4096

F8 = ml_dtypes.float8_e4m3fn
BF = ml_dtypes.bfloat16

_waitsplit_ctr = [0]


def _split_excess_waits(nc, max_waits=1):
    """This walrus build allows only `max_waits` semaphore waits per
    instruction; hoist the excess onto preceding same-engine NoOps."""
    for f in nc.m.functions:
        for blk in f.blocks:
            new = []
            for inst in blk.instructions:
                si = inst.sync_info
                if si is not None and len(si.on_wait) > max_waits:
                    waits = list(si.on_wait)
                    head, tail = waits[:-max_waits], waits[-max_waits:]
                    for s in range(0, len(head), max_waits):
                        nop = mybir.InstNoOp(
                            name=f"I-waitsplit-{_waitsplit_ctr[0]}", ins=[], outs=[])
                        _waitsplit_ctr[0] += 1
                        nop.engine = inst.engine
                        nop.sync_info = mybir.SyncInfo(
                            on_wait=list(head[s:s + max_waits]), on_update=[])
                        new.append(nop)
                    inst.sync_info = mybir.SyncInfo(
                        on_wait=list(tail), on_update=list(si.on_update))
                new.append(inst)
            blk.instructions[:] = new


def _round_f32r(a):
    """Bit-exact emulation of the device fp32r rounding (RNE to 11 explicit
    mantissa bits)."""
    v = np.ascontiguousarray(a, dtype=np.float32).view(np.uint32)
    lower = v & np.uint32(0xFFF)
    upper = v & ~np.uint32(0xFFF)
    up = (lower > 0x800) | ((lower == 0x800) & (((v >> np.uint32(12)) & 1) == 1))
    return (upper + np.where(up, np.uint32(0x1000), np.uint32(0))).view(np.float32)


def _build_nc(debug=False):
    nc = bass.Bass()
    f32 = dt.float32
    bf16 = dt.bfloat16
    fp8 = dt.float8e4
    f32r = dt.float32r
    dbg = {}
    if debug:
        for nm, shp, dtp in [("DXA", [102, T * N], bf16),
                             ("DAG01G", [128, 1024], fp8),
                             ("DAG2G", [128, N], bf16),
                             ("DZS0", [128, N], bf16),
                             ("DZS1", [128, N], bf16),
                             ("DCH", [128, N], bf16),
                             ("DZP", [128, N], bf16),
                             ("DCN", [128, N], fp8),
                             ("DAG01U", [128, 1024], fp8),
                             ("DAG2U", [128, N], bf16),
                             ("DHC", [128, N], bf16),
                             ("DHT", [128, N], bf16),
                             ("DHN", [128, N], fp8)]:
            dbg[nm] = nc.declare_dram_parameter(nm, shp, dtp, isOutput=True)

    GTP_d = nc.declare_dram_parameter("GTP", [K, 128, 4096], fp8, isOutput=False)
    XNP_d = nc.declare_dram_parameter("XNP", [128, 768], fp8, isOutput=False)
    # dtype-grouped weight blobs: one DMA each (HWDGE serializes per-DMA)
    WF8_d = nc.declare_dram_parameter("WF8", [128, 1536], fp8, isOutput=False)
    WBF_d = nc.declare_dram_parameter("WBF", [128, 576], bf16, isOutput=False)
    BIA_d = nc.declare_dram_parameter("BIA", [128, 3], f32, isOutput=False)
    H0T_d = nc.declare_dram_parameter("H0T", [2 * 128, N], bf16, isOutput=False)
    H0N_d = nc.declare_dram_parameter("H0N", [2 * 128, N], fp8, isOutput=False)
    HOUT_d = nc.declare_dram_parameter("HOUT", [2 * 128, N], bf16, isOutput=True)
    XAS_d = nc.dram_tensor("XAS_scratch", [K, BL * T * C, N], bf16)

    with tile.TileContext(nc) as tc:
        with tc.tile_pool(name="const", bufs=1) as cst, \
             tc.tile_pool(name="ag", bufs=2) as agp, \
             tc.tile_pool(name="zs", bufs=2) as zsp, \
             tc.tile_pool(name="chz", bufs=2) as chp, \
             tc.tile_pool(name="hcd", bufs=2) as hcp, \
             tc.tile_pool(name="ht", bufs=2) as htp, \
             tc.tile_pool(name="hn", bufs=2) as hnp, \
             tc.tile_pool(name="cn", bufs=2) as cnp, \
             tc.tile_pool(name="aggps", bufs=4, space="PSUM") as aggps, \
             tc.tile_pool(name="zrps", bufs=2, space="PSUM") as zrps:

            def load(dst_shape, dtp, src_ap, tag):
                d = cst.tile(dst_shape, dtp, tag=tag, name=tag)
                nc.sync.dma_start(d[:], src_ap)
                return d

            # ---- constants (time-loop-critical DMAs first) ----
            gtp = [load([128, 4096], fp8, GTP_d[0], "gtp0")]
            xnp = load([128, 768], fp8, XNP_d[:], "xnp")
            gtp += [load([128, 4096], fp8, GTP_d[k], f"gtp{k}")
                    for k in range(1, K)]
            wf8 = load([128, 1536], fp8, WF8_d[:], "wf8")
            wg01 = [wf8[:, 0:512], wf8[:, 512:1024]]
            wu01 = wf8[:, 1024:1536]
            wbf = load([128, 576], bf16, WBF_d[:], "wbf")
            wg2 = wbf[:, 0:128]
            wxg = wbf[:, 128:256]
            wu2 = wbf[:, 256:384]
            wxu = wbf[:, 384:448]
            eye8 = wbf[:, 448:576]
            bia = load([128, 3], f32, BIA_d[:], "bia")
            bg = [bia[:, 0:1], bia[:, 1:2]]
            bu2 = bia[:, 2:3]
            XA = cst.tile([102, T * N], bf16, tag="xa", name="xa")

            HT = [None, None]
            HN = [None, None]
            for p in range(2):
                ht0 = htp.tile([128, N], bf16, tag=f"ht{p}", name=f"ht{p}")
                nc.sync.dma_start(ht0[:], H0T_d[p * 128:(p + 1) * 128, :])
                HT[p] = ht0
                hn0 = hnp.tile([128, N], fp8, tag=f"hn{p}", name=f"hn{p}")
                nc.sync.dma_start(hn0[:], H0N_d[p * 128:(p + 1) * 128, :])
                HN[p] = hn0
            # ---- XA precompute: 16*(G_k @ x)^T via fp8 DR (x and G split) ----
            XPAIRS = [(0, 0), (1, 1), (2, 0), (3, 1), (0, 2), (1, 3)]
            for k in range(K):
                ps = aggps.tile([BL * T * C, N], f32, tag="agg", name=f"xps{k}")
                for i, (xc, gc) in enumerate(XPAIRS):
                    lh = xnp[:, xc * 192:(xc + 1) * 192].rearrange(
                        "p (j m) -> p j m", j=2)
                    rh = gtp[k][:, gc * 1024:(gc + 1) * 1024].rearrange(
                        "p (j n) -> p j n", j=2)
                    nc.tensor.matmul(ps[:], lh, rh, start=(i == 0),
                                     stop=(i == len(XPAIRS) - 1),
                                     perf_mode=PM.DoubleRow)
                xas = cst.tile([BL * T * C, N], bf16, tag=f"xas{k}",
                               name=f"xas{k}")
                nc.vector.tensor_copy(xas[:], ps[:])
                nc.sync.dma_start(XAS_d[k], xas[:])
            srcv = XAS_d.rearrange("k (b t c) i -> b c k t i", b=BL, c=C)
            for b in range(BL):
                for c in range(C):
                    row0 = b * 32 + c
                    dst = XA[row0:row0 + C * (K - 1) + 1:C, :].rearrange(
                        "p (t i) -> p t i", t=T)
                    nc.sync.dma_start(dst, srcv[b, c])

            # drain engines: v=DVE a=ACT g=Pool
            def copy_on(eng, dst, src, pri=-20):
                if eng == "v":
                    i = nc.vector.tensor_copy(dst, src)
                elif eng == "a":
                    i = nc.scalar.copy(dst, src)
                else:
                    i = nc.gpsimd.tensor_copy(dst, src)
                i.ins.bass_priority = pri
                return i

            def drain(spec, dst, src, pri=-20):
                """spec: 'v'/'a' full copy, or 'va'/'av' split halves."""
                if len(spec) == 1:
                    copy_on(spec, dst[:], src[:], pri)
                else:
                    copy_on(spec[0], dst[:, 0:256], src[:, 0:256], pri)
                    copy_on(spec[1], dst[:, 256:512], src[:, 256:512], pri)

            st = [dict(), dict()]

            def ph_agg(p, srcN, engs, tag01, tag2):
                """One aggregation pass over hops (G split hi+lo);
                returns (AG01 fp8, AG2 bf16)."""
                ag01 = agp.tile([128, 1024], fp8, tag=f"{tag01}{p}",
                                name=f"{tag01}{p}")
                ag2 = agp.tile([128, N], bf16, tag=f"{tag2}{p}",
                               name=f"{tag2}{p}")
                for k in (2, 0, 1):   # k2 first: its drain feeds the mid-group mm
                    ps = aggps.tile([128, N], f32, tag="agg", name=f"agg{p}{k}")
                    for c in range(4):           # (hi jp0, hi jp1, lo jp0, lo jp1)
                        lh = srcN[:, (c % 2) * 256:(c % 2 + 1) * 256].rearrange(
                            "p (j c) -> p j c", j=2)
                        rh = gtp[k][:, c * 1024:(c + 1) * 1024].rearrange(
                            "p (j n) -> p j n", j=2)
                        nc.tensor.matmul(ps[:], lh, rh, start=(c == 0),
                                         stop=(c == 3), perf_mode=PM.DoubleRow)
                    dst = ag01[:, k * N:(k + 1) * N] if k < 2 else ag2[:]
                    drain(engs[k], dst, ps)
                return ag01, ag2

            DRAIN_CFG = os.environ.get(
                "KERNEL_DRAINS", "a,v,v,a,v,v,va,va").split(",")
            # k0->ACT, k1/k2->DVE full-width; CN/HN split in halves across
            # DVE+ACT (best of swept configs)

            def ph_agg_gate(p, t):
                st[p]["agsG"] = ph_agg(p, HN[p], DRAIN_CFG[0:3], "a01g", "a2g")

            def ph_gate_w(p, t):
                ag01, ag2 = st[p]["agsG"]
                # group order: x (no deps) -> k2 -> k01 hi/lo (drain-dependent
                # mms last); op-major across bi to avoid head-of-line stalls
                zrt = []
                for bi in range(2):
                    b = p * 2 + bi
                    zr = zrps.tile([128, N], f32, tag=f"zr{p}", name=f"zr{p}{bi}")
                    nc.tensor.matmul(
                        zr[:], wxg[b * 32:b * 32 + K * C, :],
                        XA[b * 32:b * 32 + K * C, t * N:(t + 1) * N],
                        start=True, stop=False, tile_position=(b * 32, 0))
                    zrt.append(zr)
                for bi in range(2):
                    sl = slice(bi * 64, (bi + 1) * 64)
                    nc.tensor.matmul(zrt[bi][:], wg2[sl, :], ag2[sl, :],
                                     start=False, stop=False)
                for bi in range(2):
                    for lo in range(2):
                        nc.tensor.matmul(
                            zrt[bi][:],
                            wg01[bi][:, lo * 256:(lo + 1) * 256].rearrange(
                                "p (j o) -> p j o", j=2),
                            ag01[:].rearrange("p (j n) -> p j n", j=2),
                            start=False, stop=(lo == 1), perf_mode=PM.DoubleRow)
                zs = [None, None]
                for bi in range(2):
                    z = zsp.tile([128, N], bf16, tag=f"zs{p}{bi}",
                                 name=f"zs{p}{bi}")
                    nc.scalar.activation(z[:], zrt[bi][:], AF.Sigmoid,
                                         bias=bg[bi][:], scale=1.0 / S_TOT)
                    zs[bi] = z
                st[p]["zs"] = zs

            def ph_rt(p, t):
                zs = st[p]["zs"]
                # zrsig_b0 = [r|z], zrsig_b1 = [z|r]; ch rows aligned per bi
                ch = chp.tile([128, N], bf16, tag=f"ch{p}", name=f"ch{p}")
                nc.vector.tensor_tensor(ch[0:64, :], zs[0][0:64, :],
                                        HT[p][0:64, :], ALU.mult)
                nc.vector.tensor_tensor(ch[64:128, :], zs[1][64:128, :],
                                        HT[p][64:128, :], ALU.mult)
                zp = chp.tile([128, N], bf16, tag=f"zp{p}", name=f"zp{p}")
                copy_on("g", zp[0:64, :], zs[0][64:128, :], pri=0)
                copy_on("g", zp[64:128, :], zs[1][0:64, :], pri=0)
                st[p]["zp"] = zp
                st[p]["ch"] = ch
                # regular matmul against 8*I: transposes AND applies the S_H
                # scale (the PE transpose datapath ignores identity values)
                trp = zrps.tile([128, N], f32, tag=f"zr{p}", name=f"trc{p}")
                for j in range(NT):
                    sl = slice(j * 128, (j + 1) * 128)
                    nc.tensor.matmul(trp[:, sl], ch[:, sl], eye8[:],
                                     start=True, stop=True)
                cn = cnp.tile([128, N], fp8, tag=f"cn{p}", name=f"cn{p}")
                drain(DRAIN_CFG[6], cn, trp)
                st[p]["CN"] = cn
                # off-critical-path GRU precomputes: u = z*h, w = h - u
                u = hcp.tile([128, N], bf16, tag=f"u{p}", name=f"u{p}")
                nc.gpsimd.tensor_tensor(u[:], zp[:], HT[p][:], ALU.mult)
                w = hcp.tile([128, N], bf16, tag=f"w{p}", name=f"w{p}")
                nc.gpsimd.tensor_tensor(w[:], HT[p][:], u[:], ALU.subtract)
                st[p]["w"] = w

            def ph_agg_cand(p, t):
                st[p]["agsU"] = ph_agg(p, st[p]["CN"], DRAIN_CFG[3:6],
                                       "a01u", "a2u")

            def ph_upd_w(p, t):
                ag01, ag2 = st[p]["agsU"]
                hcps = zrps.tile([128, N], f32, tag=f"zr{p}", name=f"hc{p}")
                # x-part slices open (start) each half; block-diag k2 and
                # full-tile DR k01 (drain-dependent) accumulate after.
                for bi in range(2):
                    b = p * 2 + bi
                    sl = slice(bi * 64, (bi + 1) * 64)
                    nc.tensor.matmul(
                        hcps[sl, :], wxu[b * 32:b * 32 + K * C, :],
                        XA[b * 32:b * 32 + K * C, t * N:(t + 1) * N],
                        start=True, stop=False, tile_position=(b * 32, bi * 64),
                        skip_group_check=True)
                nc.tensor.matmul(hcps[:], wu2[:], ag2[:],
                                 start=False, stop=False, skip_group_check=True)
                for lo in range(2):
                    nc.tensor.matmul(
                        hcps[:],
                        wu01[:, lo * 256:(lo + 1) * 256].rearrange(
                            "p (j o) -> p j o", j=2),
                        ag01[:].rearrange("p (j n) -> p j n", j=2),
                        start=False, stop=(lo == 1), perf_mode=PM.DoubleRow,
                        skip_group_check=True)
                hc = hcp.tile([128, N], bf16, tag=f"hc{p}", name=f"hcs{p}")
                nc.scalar.activation(hc[:], hcps[:], AF.Tanh, bias=bu2[:],
                                     scale=1.0 / S_TOT)
                st[p]["hc"] = hc

            def ph_update(p, t):
                hc, zp, w = st[p]["hc"], st[p]["zp"], st[p]["w"]
                zhc = hcp.tile([128, N], bf16, tag=f"zhc{p}", name=f"zhc{p}")
                nc.vector.tensor_tensor(zhc[:], zp[:], hc[:], ALU.mult)
                htn = htp.tile([128, N], bf16, tag=f"ht{p}", name=f"htn{p}")
                nc.vector.tensor_tensor(htn[:], w[:], zhc[:], ALU.add)
                HT[p] = htn
                if t < T - 1:
                    trp = zrps.tile([128, N], f32, tag=f"zr{p}", name=f"trh{p}")
                    for j in range(NT):
                        sl = slice(j * 128, (j + 1) * 128)
                        # back-to-back per-j accumulation: w then zhc
                        # (interleaved open psum groups miscompute on HW)
                        nc.tensor.matmul(trp[:, sl], w[:, sl], eye8[:],
                                         start=True, stop=False)
                        nc.tensor.matmul(trp[:, sl], zhc[:, sl], eye8[:],
                                         start=False, stop=True)
                    hn = hnp.tile([128, N], fp8, tag=f"hn{p}", name=f"hnn{p}")
                    drain(DRAIN_CFG[7], hn, trp)
                    HN[p] = hn

            def dump(nm, src):
                if debug:
                    nc.sync.dma_start(dbg[nm][:], src[:])

            PHASES = [ph_agg_gate, ph_gate_w, ph_rt, ph_agg_cand,
                      ph_upd_w, ph_update]
            OFF = int(os.environ.get("KERNEL_OFF", "2"))
            for tick in range(6 * T + OFF):
                for p in range(2):
                    local = tick - OFF * p
                    if 0 <= local < 6 * T:
                        t, ph = divmod(local, 6)
                        PHASES[ph](p, t)
                        if debug and p == 0 and t == 0:
                            if ph == 0:
                                dump("DAG01G", st[0]["agsG"][0])
                                dump("DAG2G", st[0]["agsG"][1])
                            elif ph == 1:
                                dump("DZS0", st[0]["zs"][0])
                                dump("DZS1", st[0]["zs"][1])
                            elif ph == 2:
                                dump("DZP", st[0]["zp"])
                                dump("DCH", st[0]["ch"])
                                dump("DCN", st[0]["CN"])
                            elif ph == 3:
                                dump("DAG01U", st[0]["agsU"][0])
                                dump("DAG2U", st[0]["agsU"][1])
                            elif ph == 4:
                                dump("DHC", st[0]["hc"])
                            elif ph == 5:
                                dump("DHT", HT[0])
                                dump("DHN", HN[0])

            for p in range(2):
                nc.sync.dma_start(HOUT_d[p * 128:(p + 1) * 128, :], HT[p][:])

    _split_excess_waits(nc, max_waits=1)
    return nc


_NC_CACHE = {}


def _get_nc():
    if "nc" not in _NC_CACHE:
        _NC_CACHE["nc"] = _build_nc()
    return _NC_CACHE["nc"]


def _host_prep(G, x_seq, init_h, W_gate, b_gate, W_update, b_update):
    f32 = np.float32
    GT = np.ascontiguousarray(np.asarray(G).transpose(0, 2, 1)).astype(f32)
    GTP = np.empty((K, 128, 4096), F8)
    for k in range(K):
        gt4 = (S_G * GT[k]).reshape(2, 2, 128, N)        # [jp, jj, p, n]
        lay = gt4.transpose(2, 0, 1, 3).reshape(128, 2048)
        hi = lay.astype(F8)
        lo = (lay - hi.astype(f32)).astype(F8)
        GTP[k, :, 0:2048] = hi
        GTP[k, :, 2048:4096] = lo

    WG3 = np.asarray(W_gate, f32).reshape(K, P, 2 * H)
    WU3 = np.asarray(W_update, f32).reshape(K, P, H)
    ords = [np.r_[np.arange(H, 2 * H), np.arange(0, H)], np.arange(2 * H)]

    def split8(a):
        hi = a.astype(F8)
        return hi, (a - hi.astype(f32)).astype(F8)

    # Gate k0/k1 DR weights: full-contract [128,(2,128)] per batch parity
    # (other batch rows zero), hi half in cols 0:256, lo residual 256:512.
    WG01P0 = np.zeros((128, 512), F8)
    WG01P1 = np.zeros((128, 512), F8)
    WG2 = np.zeros((128, 128), BF)
    for bi, WP in ((0, WG01P0), (1, WG01P1)):
        o = ords[bi]
        blk = S_W * np.concatenate(
            [WG3[0, C:, :][:, o], WG3[1, C:, :][:, o]], axis=1)
        hi, lo = split8(blk)
        WP[bi * 64:(bi + 1) * 64, 0:256] = hi
        WP[bi * 64:(bi + 1) * 64, 256:512] = lo
        WG2[bi * 64:(bi + 1) * 64] = (S_W * WG3[2, C:, :][:, o]).astype(BF)

    # Update weights: both batches in one mm via block-diagonal layout.
    WU01P = np.zeros((128, 512), F8)
    WU2 = np.zeros((128, 128), BF)
    for bi in range(2):
        sl = slice(bi * 64, (bi + 1) * 64)
        for j in range(2):
            blk = S_W * WU3[j, C:, :]
            hi, lo = split8(blk)
            WU01P[sl, j * 128 + bi * 64:j * 128 + (bi + 1) * 64] = hi
            WU01P[sl, 256 + j * 128 + bi * 64:256 + j * 128 + (bi + 1) * 64] = lo
        WU2[sl, sl] = (S_W * WU3[2, C:, :]).astype(BF)

    # XA now carries the S_G=16 factor from the fp8 GTP precompute
    SX = S_TOT / S_G
    WXG = np.zeros((102, 128), BF)
    WXU = np.zeros((102, 64), BF)
    for b in range(BL):
        o = ords[b % 2]
        for k in range(K):
            r0 = b * 32 + k * C
            WXG[r0:r0 + C, :] = (SX * WG3[k, :C, :][:, o]).astype(BF)
            WXU[r0:r0 + C, :] = (SX * WU3[k, :C, :]).astype(BF)

    bg = np.asarray(b_gate, f32)
    bu = np.asarray(b_update, f32)
    WF8 = np.concatenate([WG01P0, WG01P1, WU01P], axis=1)
    WXGP = np.zeros((128, 128), BF)
    WXGP[0:102] = WXG
    WXUP = np.zeros((128, 64), BF)
    WXUP[0:102] = WXU
    WBF = np.concatenate([WG2, WXGP, WU2, WXUP,
                          (S_H * np.eye(128)).astype(BF)], axis=1)
    BIA = np.stack([bg[ords[0]], bg[ords[1]],
                    np.concatenate([bu, bu])], axis=1).astype(f32)
    shared = {"GTP": GTP, "WF8": WF8, "WBF": WBF, "BIA": BIA}
    in_maps = []
    xs_all = np.asarray(x_seq, f32)
    h0_all = np.asarray(init_h, f32)
    for core in range(NCORES):
        b0 = core * BL
        xs = xs_all[b0:b0 + BL]                          # [4, 12, 512, 2]
        h0 = h0_all[b0:b0 + BL]                          # [4, 512, 64]
        m = dict(shared)
        xnf = xs.transpose(2, 0, 1, 3).reshape(N, BL * T * C)
        xhi = xnf.astype(F8)
        xlo = (xnf - xhi.astype(f32)).astype(F8)
        xnp = np.empty((128, 768), F8)
        for lvl, xv in ((0, xhi), (1, xlo)):
            for jp in range(2):
                for jj in range(2):
                    c = lvl * 2 + jp
                    src = xv[(jp * 2 + jj) * 128:(jp * 2 + jj + 1) * 128, :]
                    xnp[:, c * 192 + jj * 96:(c * 192 + (jj + 1) * 96)] = src
        m["XNP"] = xnp
        h0t = np.empty((2 * 128, N), BF)
        h0n = np.empty((2 * 128, N), F8)
        for p in range(2):
            h0p = h0[p * 2:(p + 1) * 2]                  # [2, 512, 64]
            h0t[p * 128:(p + 1) * 128] = h0p.transpose(0, 2, 1).reshape(
                128, N).astype(BF)
            h0n[p * 128:(p + 1) * 128] = (S_H * h0p).reshape(
                2, NT, 128, H).transpose(2, 1, 0, 3).reshape(128, N).astype(F8)
        m["H0T"] = h0t
        m["H0N"] = h0n
        in_maps.append(m)
    return in_maps


def _run(inputs, trace=False):
    nc = _get_nc()
    in_maps = _host_prep(**inputs)
    res = run_bass_kernel_spmd(nc, in_maps, list(range(NCORES)), trace=trace)
    outs = []
    for core in range(NCORES):
        hout = np.asarray(res.results[core]["HOUT"]).astype(np.float32)
        per = np.empty((BL, N, H), np.float32)
        for p in range(2):
            for bi in range(2):
                r0 = p * 128 + bi * 64
                per[p * 2 + bi] = hout[r0:r0 + 64, :].T
        outs.append(per)
    full = np.concatenate(outs, axis=0).astype(np.float32)
    return full, res


def kernel(G, x_seq, init_h, W_gate, b_gate, W_update, b_update):
    full, _ = _run(dict(G=G, x_seq=x_seq, init_h=init_h, W_gate=W_gate,
                        b_gate=b_gate, W_update=W_update, b_update=b_update))
    return full
